# revision 4
# baseline (speedup 1.0000x reference)
"""Bass/Trainium2 kernel for nn_BipartPool (bipartite GATv2 pooling).

Math (per graph g, centroid r, head h):
  logit[s,r,h] = att_h . leaky_relu(x_l[s] + x_r[r], 0.2)
              = 0.8*att_h . relu(v) + 0.2*att_h . v        (v = x_l[s]+x_r[r])
  The 0.2*att.x_r term is constant per softmax group -> cancels; dropped.
  The 0.2*att.x_l term is rank-1 over (s,h) -> folded as one extra
  accumulating matmul (att_al) per graph.
  Softmax computed unnormalized (logits are O(+-6), exp is safe in fp32);
  the denominator comes for free as a ones-column in the aggregation matmul.

Self-loops: PyG adds (i,i) for i < NT; these are cross-graph. Each core gets
x[256k:256k+256] as `xloop`. For t < 16 the reference masks the duplicate
dense edge (s==t) but re-adds it as a self-loop with an identical logit, so
we keep the dense edge and kill the loop edge via a -1e30 bias (core 0 only).

Sharding: 8 cores x 16 graphs (8192 source rows, 256 targets each).
"""

import os
import sys
import numpy as np

sys.path.insert(0, "/opt/trn_rl_repo")

N = 65536
R = 16
B = 128
IN = 64
H = 2
C = 64
NT = R * B
NCORES = 8
SG = B // NCORES          # 16 graphs per core
S = N // NCORES           # 8192 source rows per core
NS = N // B               # 512 sources per graph
T = NT // NCORES          # 256 targets per core
HC = H * C                # 128

_nc_cache = {}


def _build_nc():
    if "nc" in _nc_cache:
        return _nc_cache["nc"]
    from concourse import bacc, mybir
    import concourse.bass as bass
    import concourse.tile as tile

    f32 = mybir.dt.float32
    AF = mybir.ActivationFunctionType
    OP = mybir.AluOpType

    nc = bacc.Bacc("TRN2", target_bir_lowering=False, debug=False)

    ins = {}
    for name, shape in [
        ("xs", [S, IN]),
        ("xloop", [T, IN]),
        ("wl_ext", [IN + 1, HC]),
        ("wla_ext", [IN + 1, HC + 1]),
        ("attr", [HC, 512]),
        ("att_al", [HC, 32]),
        ("att_blk", [HC, 2]),
        ("att_blk_q", [HC, 2]),
        ("ones2", [1, 2]),
        ("xr_t", [HC, R]),
        ("lb", [1, T]),
        ("ident", [128, 128]),
        ("ones_row", [1, S]),
        ("bias_t", [128, C]),
        ("d01h0", [16, 32]),
        ("d01h1", [16, 32]),
        ("sel0", [32, 16]),
        ("sel1", [32, 16]),
    ]:
        ins[name] = nc.dram_tensor(name, shape, f32, kind="ExternalInput").ap()
    out_d = nc.dram_tensor("out", [T, C], f32, kind="ExternalOutput").ap()

    with tile.TileContext(nc) as tc:
        with (
            tc.tile_pool(name="const", bufs=1) as cpool,
            tc.tile_pool(name="big", bufs=1) as bpool,
            tc.tile_pool(name="work", bufs=3) as wpool,
            tc.tile_pool(name="epool", bufs=6) as epool,
            tc.tile_pool(name="ps", bufs=2, space="PSUM") as pspool,
            tc.tile_pool(name="ps1", bufs=1, space="PSUM") as pspool1,
        ):
            # ---- constants to SBUF
            cs = {}
            for name in [
                "wl_ext", "wla_ext", "attr", "att_al", "att_blk",
                "att_blk_q", "ones2", "xr_t", "lb", "ident", "bias_t",
                "d01h0", "d01h1", "sel0", "sel1",
            ]:
                shp = list(ins[name].shape)
                t = cpool.tile(shp, f32, tag=name)
                nc.sync.dma_start(t[:], ins[name][:])
                cs[name] = t

            xt = bpool.tile([IN + 1, S], f32, tag="xt")
            xlT = bpool.tile([HC, S], f32, tag="xlT")
            xla = bpool.tile([HC, 64 * (HC + 1)], f32, tag="xla")
            nc.sync.dma_start(xt[IN:IN + 1, :], ins["ones_row"][:])

            # ---- phase 1: transpose x into xt (2 source-tiles per PE shot)
            for i in range(32):
                tin = wpool.tile([128, 128], f32, tag="tin")
                src = ins["xs"][256 * i:256 * i + 256, :]
                nc.sync.dma_start(
                    tin[:], src.rearrange("(a p) c -> p a c", p=128)
                )
                tp = pspool.tile([128, 128], f32, tag="tpo")
                nc.tensor.transpose(tp[:], tin[:], cs["ident"][:])
                eng = nc.scalar if i % 2 == 0 else nc.vector
                if i % 2 == 0:
                    nc.scalar.copy(xt[0:64, 256 * i:256 * i + 128], tp[0:64, :])
                    nc.vector.tensor_copy(
                        xt[0:64, 256 * i + 128:256 * i + 256], tp[64:128, :])
                else:
                    nc.vector.tensor_copy(
                        xt[0:64, 256 * i:256 * i + 128], tp[0:64, :])
                    nc.scalar.copy(
                        xt[0:64, 256 * i + 128:256 * i + 256], tp[64:128, :])

            # ---- phase 1b: x_l^T = (W_ext^T @ xt)  [HC, S]
            for c in range(16):
                ps = pspool.tile([128, 512], f32, tag="xl")
                nc.tensor.matmul(
                    ps[:], cs["wl_ext"][:], xt[:, 512 * c:512 * c + 512],
                    start=True, stop=True)
                if c % 2 == 0:
                    nc.scalar.copy(xlT[:, 512 * c:512 * c + 512], ps[:])
                else:
                    nc.vector.tensor_copy(xlT[:, 512 * c:512 * c + 512], ps[:])

            # ---- phase 1c: x_lA = [x@W+b | 1]  (s on partitions), 64 chunks
            for i in range(64):
                ps = pspool.tile([128, HC + 1], f32, tag="xl")
                nc.tensor.matmul(
                    ps[:], xt[:, 128 * i:128 * i + 128], cs["wla_ext"][:],
                    start=True, stop=True)
                o = (HC + 1) * i
                if i % 2 == 0:
                    nc.scalar.copy(xla[:, o:o + HC + 1], ps[:])
                else:
                    nc.vector.tensor_copy(xla[:, o:o + HC + 1], ps[:])

            # ---- phase 1d: loop-edge prep
            xt_loop = bpool.tile([IN + 1, T], f32, tag="xt_loop")
            nc.sync.dma_start(xt_loop[IN:IN + 1, :], ins["ones_row"][0:1, 0:T])
            tinL = wpool.tile([128, 128], f32, tag="tin")
            nc.sync.dma_start(
                tinL[:], ins["xloop"][:].rearrange("(a p) c -> p a c", p=128))
            tpL = pspool.tile([128, 128], f32, tag="tpo")
            nc.tensor.transpose(tpL[:], tinL[:], cs["ident"][:])
            nc.scalar.copy(xt_loop[0:64, 0:128], tpL[0:64, :])
            nc.vector.tensor_copy(xt_loop[0:64, 128:256], tpL[64:128, :])

            xlloopT = bpool.tile([HC, T], f32, tag="xlloopT")
            psL = pspool.tile([128, 512], f32, tag="xl")
            nc.tensor.matmul(psL[:, 0:T], cs["wl_ext"][:], xt_loop[:],
                             start=True, stop=True)
            nc.scalar.copy(xlloopT[:], psL[:, 0:T])

            # per-graph loop-source features [x_l|1], partitions 0..15
            xlloopA = bpool.tile([16, 16 * (HC + 1)], f32, tag="xlloopA")
            for g in range(16):
                psA = pspool.tile([128, HC + 1], f32, tag="xl")
                nc.tensor.matmul(
                    psA[0:16, :], xt_loop[:, 16 * g:16 * g + 16],
                    cs["wla_ext"][:], start=True, stop=True)
                eng = nc.vector.tensor_copy if g % 2 else nc.scalar.copy
                eng(xlloopA[:, (HC + 1) * g:(HC + 1) * (g + 1)], psA[0:16, :])

            # E_loop = relu(xlloopT + x_r[t % 16]) via 16 strided tensor_scalar
            E_loop = bpool.tile([HC, T], f32, tag="E_loop")
            xv = xlloopT[:].rearrange("p (g r) -> p r g", r=R)
            ev = E_loop[:].rearrange("p (g r) -> p r g", r=R)
            for r in range(R):
                nc.vector.tensor_scalar(
                    out=ev[:, r, :], in0=xv[:, r, :],
                    scalar1=cs["xr_t"][:, r:r + 1], scalar2=0.0,
                    op0=OP.add, op1=OP.max)

            zL = pspool.tile([32, 512], f32, tag="z")
            nc.tensor.matmul(zL[0:2, 0:T], cs["att_blk"][:], E_loop[:],
                             start=True, stop=False)
            nc.tensor.matmul(zL[0:2, 0:T], cs["att_blk_q"][:], xlloopT[:],
                             start=False, stop=False)
            nc.tensor.matmul(zL[0:2, 0:T], cs["ones2"][:], cs["lb"][:],
                             start=False, stop=True)
            p_loop = bpool.tile([2, T], f32, tag="p_loop")
            nc.scalar.activation(p_loop[:], zL[0:2, 0:T], AF.Exp, scale=0.8)

            # per-graph transpose of p_loop -> plg_sb[16, 2g+h]
            plps = pspool.tile([16, 32], f32, tag="tpo")
            for g in range(16):
                nc.tensor.transpose(
                    plps[:, 2 * g:2 * g + 2], p_loop[:, 16 * g:16 * g + 16],
                    cs["ident"][0:2, 0:2])
            plg = bpool.tile([16, 32], f32, tag="plg")
            nc.vector.tensor_copy(plg[:], plps[:])

            out_all = bpool.tile([16, 16 * C], f32, tag="out_all")

            # ---- phase 2: per-graph attention + softmax + aggregation
            for g in range(16):
                Es = []
                for r in range(R):
                    E = epool.tile([HC, NS], f32, tag="E")
                    src = xlT[:, NS * g:NS * (g + 1)]
                    bcol = cs["xr_t"][:, r:r + 1]
                    if r % 8 < 5:
                        nc.vector.tensor_scalar(
                            out=E[:], in0=src, scalar1=bcol, scalar2=0.0,
                            op0=OP.add, op1=OP.max)
                    else:
                        nc.scalar.activation(E[:], src, AF.Relu, bias=bcol)
                    Es.append(E)
                z = pspool.tile([32, 512], f32, tag="z")
                for r in range(R):
                    nc.tensor.matmul(
                        z[:], cs["attr"][:, 32 * r:32 * r + 32], Es[r][:],
                        start=(r == 0), stop=False)
                nc.tensor.matmul(
                    z[:], cs["att_al"][:], xlT[:, NS * g:NS * (g + 1)],
                    start=False, stop=True)
                P = wpool.tile([32, NS], f32, tag="P")
                nc.scalar.activation(P[:], z[:], AF.Exp, scale=0.8)

                ptp = pspool.tile([128, 128], f32, tag="tpo")
                for c in range(4):
                    nc.tensor.transpose(
                        ptp[:, 32 * c:32 * c + 32],
                        P[:, 128 * c:128 * (c + 1)], cs["ident"][0:32, 0:32])
                pTs = wpool.tile([128, 128], f32, tag="pT")
                if g % 2 == 0:
                    nc.scalar.copy(pTs[:], ptp[:])
                else:
                    nc.vector.tensor_copy(pTs[:], ptp[:])

                # loop-edge diagonal lhsT: diag[r, 16h+r] = p_loop(g, r, h)
                d0 = wpool.tile([16, 32], f32, tag="d0")
                nc.vector.tensor_scalar(
                    out=d0[:], in0=cs["d01h0"][:], scalar1=plg[:, 2 * g:2 * g + 1],
                    scalar2=None, op0=OP.mult)
                d1 = wpool.tile([16, 32], f32, tag="d1")
                nc.vector.tensor_scalar(
                    out=d1[:], in0=cs["d01h1"][:],
                    scalar1=plg[:, 2 * g + 1:2 * g + 2],
                    scalar2=None, op0=OP.mult)
                dg = wpool.tile([16, 32], f32, tag="dg")
                nc.vector.tensor_tensor(dg[:], d0[:], d1[:], OP.add)

                agg = pspool1.tile([32, HC + 1], f32, tag="agg")
                for c in range(4):
                    nc.tensor.matmul(
                        agg[:], pTs[:, 32 * c:32 * c + 32],
                        xla[:, (HC + 1) * (4 * g + c):(HC + 1) * (4 * g + c + 1)],
                        start=(c == 0), stop=False)
                nc.tensor.matmul(
                    agg[:], dg[:],
                    xlloopA[:, (HC + 1) * g:(HC + 1) * (g + 1)],
                    start=False, stop=True)

                rec = wpool.tile([32, 1], f32, tag="rec")
                nc.vector.reciprocal(rec[:], agg[:, HC:HC + 1])
                rec2 = wpool.tile([32, 1], f32, tag="rec2")
                nc.scalar.mul(rec2[:], rec[:], 0.5)
                Un = wpool.tile([32, HC], f32, tag="Un")
                nc.vector.tensor_scalar(
                    out=Un[:], in0=agg[:, 0:HC], scalar1=rec2[:], scalar2=None,
                    op0=OP.mult)
                og = pspool1.tile([16, C], f32, tag="og")
                nc.tensor.matmul(og[:], cs["sel0"][:], Un[:, 0:C],
                                 start=True, stop=False)
                nc.tensor.matmul(og[:], cs["sel1"][:], Un[:, C:2 * C],
                                 start=False, stop=True)
                nc.vector.tensor_tensor(
                    out_all[:, C * g:C * (g + 1)], og[:], cs["bias_t"][0:16, :],
                    OP.add)

            nc.sync.dma_start(
                out_d.rearrange("(g p) c -> p g c", p=16),
                out_all[:].rearrange("p (g c) -> p g c", g=16))

    nc.compile()
    _nc_cache["nc"] = nc
    return nc


def _host_inputs(x, xcent_base, lin_l_w, lin_l_b, lin_r_w, lin_r_b, att, bias):
    f = np.float32
    x = np.asarray(x, f)
    att = np.asarray(att, f)
    att_blk = np.zeros((HC, 2), f)
    for h in range(H):
        att_blk[64 * h:64 * (h + 1), h] = att[h]
    attr = np.zeros((HC, 512), f)
    for r in range(R):
        for h in range(H):
            attr[:, 32 * r + 16 * h + r] = att_blk[:, h]
    att_al = np.zeros((HC, 32), f)
    for r in range(R):
        for h in range(H):
            att_al[:, 16 * h + r] = 0.25 * att_blk[:, h]
    wl_ext = np.concatenate(
        [np.asarray(lin_l_w, f), np.asarray(lin_l_b, f)[None, :]], axis=0)
    wla_ext = np.zeros((IN + 1, HC + 1), f)
    wla_ext[:, :HC] = wl_ext
    wla_ext[IN, HC] = 1.0
    xr_t = np.ascontiguousarray(
        (np.asarray(xcent_base, f) @ np.asarray(lin_r_w, f)
         + np.asarray(lin_r_b, f)).T.astype(f))
    common = dict(
        wl_ext=wl_ext,
        wla_ext=wla_ext,
        attr=attr,
        att_al=att_al,
        att_blk=att_blk,
        att_blk_q=(0.25 * att_blk).astype(f),
        ones2=np.ones((1, 2), f),
        xr_t=xr_t,
        ident=np.eye(128, dtype=f),
        ones_row=np.ones((1, S), f),
        bias_t=np.tile(np.asarray(bias, f)[None, :], (128, 1)),
        d01h0=np.zeros((16, 32), f),
        d01h1=np.zeros((16, 32), f),
        sel0=np.zeros((32, 16), f),
        sel1=np.zeros((32, 16), f),
    )
    for r in range(R):
        common["d01h0"][r, r] = 1.0
        common["d01h1"][r, 16 + r] = 1.0
        common["sel0"][r, r] = 1.0
        common["sel1"][16 + r, r] = 1.0
    maps = []
    for k in range(NCORES):
        lb = np.zeros((1, T), f)
        if k == 0:
            lb[0, :16] = -1e30
        m = dict(common)
        m["xs"] = np.ascontiguousarray(x[S * k:S * (k + 1)])
        m["xloop"] = np.ascontiguousarray(x[T * k:T * (k + 1)])
        m["lb"] = lb
        maps.append(m)
    return maps


def kernel(x, edge_index, batch, xcent_base, lin_l_w, lin_l_b, lin_r_w,
           lin_r_b, att, bias, **_unused):
    from concourse.bass_utils import run_bass_kernel_spmd

    nc = _build_nc()
    in_maps = _host_inputs(x, xcent_base, lin_l_w, lin_l_b, lin_r_w, lin_r_b,
                           att, bias)
    res = run_bass_kernel_spmd(nc, in_maps, list(range(NCORES)))
    out = np.concatenate([res.results[k]["out"] for k in range(NCORES)], axis=0)
    return out.astype(np.float32)
